# revision 6
# baseline (speedup 1.0000x reference)
"""Trainium2 Bass kernel: 7x7 valid 2D cross-correlation on a 6144x6144 fp32
image, + scalar bias. Output 6138x6138 fp32.

Strategy (t32 scheme, v5)
-------------------------
Row-band sharding across 8 NeuronCores: core c computes output rows
[c*768, c*768+768) for all 6138 output columns. Inputs stream as fp16
(rel err ~5e-4, well inside the 2e-2 gate); output is stored fp16 and
upcast on host.

Per core the conv runs as 16 CONCURRENT 32x32-tile banded matmuls on the
PE array (4 row-groups x 4 col-groups, tile_position packing):

  - output rows are split into 26-row blocks; block k lives in SBUF
    partition group (k mod 4) of a 128-partition window tile holding
    input rows [104w + 26b, 104w + 26b + 32) (26 outputs + 6 halo).
  - stationary for tap j, block-group b: A[k, m] = W[k - m, j]
    (0 <= k-m < 7, m < 26), a [32, 26] band at array tile (32b, 32g).
  - col-group g processes output-column quarter [2048c + 512g, +512);
    its matmul streams the j-shifted AP and writes PSUM partitions
    [32g, 32g+26) of bank b.  Issue order is j-major, g middle, b inner:
    consecutive matmuls hit different col-groups (independent XBUS
    streams) and different row-groups (LDWEIGHTS pull-ahead), which is
    what lets the 16 tiles overlap at ~34 ns/matmul sustained.
  - 7 taps accumulate per bank; one round (w, c) = 112 matmuls into a
    4-bank psum tile ps4 [128, 2048], double-buffered (8 banks total).

I/O path: every DMA instruction costs ~0.6-0.9us of sequencer DGE-setup
time regardless of size, and one instruction's descriptors fan out over
only ~4 DMA queues, so both too-many and too-few DMA instructions lose.

  - loads: one 2-dim DMA per 32-row block (4/window) on the SP ring;
    window 0 is split at col 2054 so chunk-0 matmuls start early.
  - evictions (fp32 PSUM -> fp16 SBUF + bias): split DVE (cols 0:1024)
    / ACT (cols 1024:2048) to halve the latency, which sits on the
    ps4-double-buffer critical path between rounds.
  - stores: 4 per round (one 3-level-AP DMA per col-group quarter g:
    26 rows x 4 blocks x 512 cols), alternating ACT/SP rings so neither
    sequencer exceeds the ~3.8us round cadence.  (GpSimd software-DGE
    stores at ~680ns/call serializing on the engine were the original
    v1 bottleneck: stores fell behind and drained ~47us after the last
    matmul.)
"""

import os

import numpy as np

import concourse.tile as tile
from concourse import bacc, mybir
from concourse.ap import AP
from concourse.bass_utils import run_bass_kernel_spmd

H = 6144
W = 6144
KH = 7
KW = 7
OH = H - KH + 1          # 6138
OW = W - KW + 1          # 6138
NCORES = 8
RPC = 768                # output rows per core (8*768 = 6144; last 6 dropped)
IRPC = RPC + KH - 1      # 774 input rows per core

# t32 scheme geometry
BK = 26                  # output rows per block (32-row tile - 6 halo)
NBLK = (RPC + BK - 1) // BK   # 30 blocks (last has 768 - 29*26 = 14 rows)
WSTEP = 4 * BK           # 104 input rows consumed per window
NWIN = (NBLK + 3) // 4   # 8 windows (last has 2 blocks)
CHW = 2048               # output columns per chunk (4 col-group quarters)
NCH = 3                  # chunks: cols [0,2048),[2048,4096),[4096,6144)
XW = W + 12              # window tile cols: 6144 + zero halo (max read col
                         # is 4096+1536+6+512 = 6150)
YROWS = WSTEP * (NWIN - 1) + 2 * BK    # 780 y staging rows (uniform 26-row
                         # block stores; host trims to 768)

_NC_CACHE = {}
LAST_RESULTS = None      # for the local test harness; the grader ignores this


def _build_nc_t32():
    f32 = mybir.dt.float32
    f16 = mybir.dt.float16

    nc = bacc.Bacc(trn_type="TRN2", target_bir_lowering=False, debug=False,
                   num_devices=NCORES)
    x = nc.dram_tensor("x", [IRPC, W], f16, kind="ExternalInput")
    bands = nc.dram_tensor("bands", [128, KW * 32], f16, kind="ExternalInput")
    bcol = nc.dram_tensor("bcol", [128, 1], f32, kind="ExternalInput")
    y = nc.dram_tensor("y", [YROWS, W], f16, kind="ExternalOutput")

    with tile.TileContext(nc) as tc:
        with tc.tile_pool(name="const", bufs=1) as constp, \
             tc.tile_pool(name="xw", bufs=8) as xp, \
             tc.tile_pool(name="psum", bufs=8, space="PSUM") as pp, \
             tc.tile_pool(name="outs", bufs=4) as op:
            bands_sb = constp.tile([128, KW * 32], f16)
            nc.sync.dma_start(bands_sb[:], bands[:])
            bcol_sb = constp.tile([128, 1], f32)
            nc.sync.dma_start(bcol_sb[:], bcol[:])

            # Warm-up burst so the PE HAM clock-gate reaches 8/8 while the
            # first window is still loading.
            dummy = constp.tile([128, 512], f16)
            nc.gpsimd.memset(dummy[:], 0.0)
            warm_ps = pp.tile([128, 4 * 512], f32, tag="ps4", bufs=2)
            for i in range(8):
                nc.tensor.matmul(warm_ps[:, 0:512], dummy[:, 0:128], dummy[:, 0:512],
                                 start=(i == 0), stop=(i == 7))

            # Window loads: one 2-dim DMA per block on the SP ring.
            xw_t = [None] * NWIN
            for w in range(NWIN):
                nblk_w = min(4, NBLK - 4 * w)
                xw = xp.tile([128, XW], f16)
                nc.gpsimd.memset(xw[:, W:XW], 0.0)
                if w == 0:
                    for b in range(nblk_w):
                        nc.sync.dma_start(xw[32 * b:32 * b + 32, 0:CHW + 6],
                                          x[BK * b:BK * b + 32, 0:CHW + 6])
                    for b in range(nblk_w):
                        nc.sync.dma_start(xw[32 * b:32 * b + 32, CHW + 6:W],
                                          x[BK * b:BK * b + 32, CHW + 6:W])
                else:
                    for b in range(nblk_w):
                        r0 = WSTEP * w + BK * b
                        rn = min(32, IRPC - r0)
                        nc.sync.dma_start(xw[32 * b:32 * b + rn, 0:W],
                                          x[r0:r0 + rn, 0:W])
                xw_t[w] = xw

            # Compute + evict + store, (window, chunk)-major.
            for w in range(NWIN):
                nblk_w = min(4, NBLK - 4 * w)
                for c in range(NCH):
                    srcx = xw_t[w]
                    ps4 = pp.tile([128, 4 * 512], f32, name=f"ps4_{w}_{c}",
                                  tag="ps4", bufs=2)
                    for j in range(KW):
                        for g in range(4):
                            for b in range(nblk_w):
                                nc.tensor.matmul(
                                    ps4[32 * g:32 * g + BK,
                                        512 * b:512 * b + 512],
                                    bands_sb[32 * b:32 * b + 32,
                                             32 * j:32 * j + BK],
                                    srcx[32 * b:32 * b + 32,
                                         CHW * c + 512 * g + j:
                                         CHW * c + 512 * g + j + 512],
                                    start=(j == 0), stop=(j == KW - 1),
                                    tile_position=(32 * b, 32 * g),
                                    skip_group_check=True)
                    ot = op.tile([128, 4 * 512], f16, name=f"ot{w}_{c}",
                                 tag="ot")
                    cw = 512 * nblk_w
                    nc.vector.tensor_scalar_add(ot[:, 0:cw // 2],
                                                ps4[:, 0:cw // 2],
                                                bcol_sb[:])
                    nc.scalar.activation(
                        ot[:, cw // 2:cw], ps4[:, cw // 2:cw],
                        mybir.ActivationFunctionType.Identity,
                        bias=bcol_sb[:])
                    # Stores: one 3-level-AP DMA per col-group quarter g
                    # (partition 32g+m, col 512b+n -> y[26(4w+b)+m,
                    # 2048c+512g+n]), alternating ACT/SP rings.
                    for g in range(4):
                        src_ap = ot[32 * g:32 * g + BK, :]
                        src3 = AP(src_ap.tensor, src_ap.offset,
                                  [[4 * 512, BK], [512, nblk_w], [1, 512]])
                        dst3 = AP(y[:, :].tensor,
                                  WSTEP * w * W + CHW * c + 512 * g,
                                  [[W, BK], [BK * W, nblk_w], [1, 512]])
                        eng = nc.scalar if g % 2 == 0 else nc.gpsimd
                        eng.dma_start(dst3, src3)
    nc.compile()
    return nc


def _get_nc(dtype_key: str):
    if dtype_key not in _NC_CACHE:
        _NC_CACHE[dtype_key] = _build_nc_t32()
    return _NC_CACHE[dtype_key]


def _build_bands_t32(weight: np.ndarray) -> np.ndarray:
    """bands[32b + k, 32j + m] = weight[k-m, j], 0 <= k-m < KH, m < 26."""
    bands = np.zeros((128, KW * 32), dtype=np.float32)
    m = np.arange(BK)
    for j in range(KW):
        for d in range(KH):
            bands[m + d, 32 * j + m] = np.float32(weight[d, j])
    for b in range(1, 4):
        bands[32 * b:32 * b + 32, :] = bands[0:32, :]
    return bands


def kernel(x: np.ndarray, weight: np.ndarray, bias: np.ndarray) -> np.ndarray:
    global LAST_RESULTS
    trace = os.environ.get("CONV_TRACE", "") == "1"

    xs = np.asarray(x, dtype=np.float32)
    assert xs.shape == (H, W), xs.shape
    wf = np.asarray(weight, dtype=np.float32)
    bands = _build_bands_t32(wf).astype(np.float16)
    bcol = np.full((128, 1), np.float32(np.asarray(bias).reshape(-1)[0]),
                   dtype=np.float32)

    xpad = np.zeros((NCORES * RPC + KH - 1, W), dtype=np.float16)
    xpad[:H, :] = xs.astype(np.float16)
    in_maps = []
    for c in range(NCORES):
        xc = np.ascontiguousarray(xpad[c * RPC:c * RPC + IRPC, :])
        in_maps.append({"x": xc, "bands": bands, "bcol": bcol})

    nc = _get_nc("t32")
    kwargs = {}
    if trace:
        kwargs = dict(trace=True, trace_cores=[0])
    res = run_bass_kernel_spmd(nc, in_maps, core_ids=list(range(NCORES)),
                               **kwargs)
    LAST_RESULTS = res
    out = np.concatenate([r["y"][:RPC] for r in res.results], axis=0)[:OH, :OW]
    return np.ascontiguousarray(out.astype(np.float32))


# revision 9
# speedup vs baseline: 1.1721x; 1.1721x over previous
"""Trainium2 Bass kernel: 7x7 valid 2D cross-correlation on a 6144x6144 fp32
image, + scalar bias. Output 6138x6138 fp32.

Strategy (t32 scheme, v5)
-------------------------
Row-band sharding across 8 NeuronCores: core c computes output rows
[c*768, c*768+768) for all 6138 output columns. Inputs stream as fp16
(rel err ~5e-4, well inside the 2e-2 gate); output is stored fp16 and
upcast on host.

Per core the conv runs as 16 CONCURRENT 32x32-tile banded matmuls on the
PE array (4 row-groups x 4 col-groups, tile_position packing):

  - output rows are split into 26-row blocks; block k lives in SBUF
    partition group (k mod 4) of a 128-partition window tile holding
    input rows [104w + 26b, 104w + 26b + 32) (26 outputs + 6 halo).
  - stationary for tap j, block-group b: A[k, m] = W[k - m, j]
    (0 <= k-m < 7, m < 26), a [32, 26] band at array tile (32b, 32g).
  - col-group g processes output-column quarter [2048c + 512g, +512);
    its matmul streams the j-shifted AP and writes PSUM partitions
    [32g, 32g+26) of bank b.  Issue order is j-major, g middle, b inner:
    consecutive matmuls hit different col-groups (independent XBUS
    streams) and different row-groups (LDWEIGHTS pull-ahead), which is
    what lets the 16 tiles overlap at ~34 ns/matmul sustained.
  - 7 taps accumulate per bank; one round (w, c) = 112 matmuls into a
    4-bank psum tile ps4 [128, 2048], double-buffered (8 banks total).

I/O path: every DMA instruction costs ~0.6-0.9us of sequencer DGE-setup
time regardless of size, and one instruction's descriptors fan out over
only ~4 DMA queues, so both too-many and too-few DMA instructions lose.

  - loads: one 2-dim DMA per 32-row block (4/window) on the SP ring;
    window 0 is split at col 2054 so chunk-0 matmuls start early.
  - evictions (fp32 PSUM -> fp16 SBUF + bias): split DVE (cols 0:1024)
    / ACT (cols 1024:2048) to halve the latency, which sits on the
    ps4-double-buffer critical path between rounds.
  - stores: 4 per round (one 3-level-AP DMA per col-group quarter g:
    26 rows x 4 blocks x 512 cols), alternating ACT/SP rings so neither
    sequencer exceeds the ~3.8us round cadence.  (GpSimd software-DGE
    stores at ~680ns/call serializing on the engine were the original
    v1 bottleneck: stores fell behind and drained ~47us after the last
    matmul.)
"""

import os

import numpy as np

import concourse.tile as tile
from concourse import bacc, mybir
from concourse.ap import AP
from concourse.bass_utils import run_bass_kernel_spmd

H = 6144
W = 6144
KH = 7
KW = 7
OH = H - KH + 1          # 6138
OW = W - KW + 1          # 6138
NCORES = 8
RPC = 768                # output rows per core (8*768 = 6144; last 6 dropped)
IRPC = RPC + KH - 1      # 774 input rows per core

# t32 scheme geometry
BK = 26                  # output rows per block (32-row tile - 6 halo)
NBLK = (RPC + BK - 1) // BK   # 30 blocks (last has 768 - 29*26 = 14 rows)
WSTEP = 4 * BK           # 104 input rows consumed per window
NWIN = (NBLK + 3) // 4   # 8 windows (last has 2 blocks)
CHW = 2048               # output columns per chunk (4 col-group quarters)
NCH = 3                  # chunks: cols [0,2048),[2048,4096),[4096,6144)
XW = W + 12              # window tile cols: 6144 + zero halo (max read col
                         # is 4096+1536+6+512 = 6150)
YROWS = WSTEP * (NWIN - 1) + 2 * BK    # 780 y staging rows (uniform 26-row
                         # block stores; host trims to 768)

_NC_CACHE = {}
LAST_RESULTS = None      # for the local test harness; the grader ignores this


def _build_nc_t32():
    f32 = mybir.dt.float32
    f16 = mybir.dt.float16

    nc = bacc.Bacc(trn_type="TRN2", target_bir_lowering=False, debug=False,
                   num_devices=NCORES)
    x = nc.dram_tensor("x", [IRPC, W], f16, kind="ExternalInput")
    bands = nc.dram_tensor("bands", [128, KW * 32], f16, kind="ExternalInput")
    bcol = nc.dram_tensor("bcol", [128, 1], f32, kind="ExternalInput")
    y = nc.dram_tensor("y", [YROWS, W], f16, kind="ExternalOutput")

    with tile.TileContext(nc) as tc:
        with tc.tile_pool(name="const", bufs=1) as constp, \
             tc.tile_pool(name="xw", bufs=8) as xp, \
             tc.tile_pool(name="psum", bufs=8, space="PSUM") as pp, \
             tc.tile_pool(name="outs", bufs=4) as op:
            bands_sb = constp.tile([128, KW * 32], f16)
            nc.sync.dma_start(bands_sb[:], bands[:])
            bcol_sb = constp.tile([128, 1], f32)
            nc.sync.dma_start(bcol_sb[:], bcol[:])

            # Warm-up burst so the PE HAM clock-gate reaches 8/8 while the
            # first window is still loading.
            dummy = constp.tile([128, 512], f16)
            nc.gpsimd.memset(dummy[:], 0.0)
            warm_ps = pp.tile([128, 4 * 512], f32, tag="ps4", bufs=2)
            for i in range(10):
                nc.tensor.matmul(warm_ps[:, 0:512], dummy[:, 0:128], dummy[:, 0:512],
                                 start=(i == 0), stop=(i == 9))

            # Window loads: one 2-dim DMA per block on the SP ring.
            # bufs=8 keeps the whole input resident; w0-w2 load up front,
            # w3+ are emitted inside the compute loop so SP-ring stores
            # are not queued behind them in program order.
            xw_t = [None] * NWIN

            def load_window(w):
                nblk_w = min(4, NBLK - 4 * w)
                xw = xp.tile([128, XW], f16)
                nc.gpsimd.memset(xw[:, W:XW], 0.0)
                if w == 0:
                    for b in range(nblk_w):
                        nc.sync.dma_start(xw[32 * b:32 * b + 32, 0:CHW + 6],
                                          x[BK * b:BK * b + 32, 0:CHW + 6])
                    for b in range(nblk_w):
                        nc.sync.dma_start(xw[32 * b:32 * b + 32, CHW + 6:W],
                                          x[BK * b:BK * b + 32, CHW + 6:W])
                else:
                    for b in range(nblk_w):
                        r0 = WSTEP * w + BK * b
                        rn = min(32, IRPC - r0)
                        if rn < 32:
                            # zero the block before the partial load: the
                            # unloaded halo rows otherwise keep fresh-SBUF
                            # bits that can be NaN, and 0 * NaN = NaN.
                            # (engine memsets need 32-aligned partitions)
                            nc.gpsimd.memset(xw[32 * b:32 * b + 32, 0:W],
                                             0.0)
                        nc.sync.dma_start(xw[32 * b:32 * b + rn, 0:W],
                                          x[r0:r0 + rn, 0:W])
                xw_t[w] = xw

            for w in range(3):
                load_window(w)

            # Compute + evict + store, (window, chunk)-major.
            for w in range(NWIN):
                nblk_w = min(4, NBLK - 4 * w)
                for c in range(NCH):
                    srcx = xw_t[w]
                    ps4 = pp.tile([128, 4 * 512], f32, name=f"ps4_{w}_{c}",
                                  tag="ps4", bufs=2)
                    for j in range(KW):
                        for g in range(4):
                            for b in range(nblk_w):
                                nc.tensor.matmul(
                                    ps4[32 * g:32 * g + BK,
                                        512 * b:512 * b + 512],
                                    bands_sb[32 * b:32 * b + 32,
                                             32 * j:32 * j + BK],
                                    srcx[32 * b:32 * b + 32,
                                         CHW * c + 512 * g + j:
                                         CHW * c + 512 * g + j + 512],
                                    start=(j == 0), stop=(j == KW - 1),
                                    tile_position=(32 * b, 32 * g),
                                    skip_group_check=True)
                    if c == 0 and 3 <= w + 2 < NWIN:
                        load_window(w + 2)
                    ot = op.tile([128, 4 * 512], f16, name=f"ot{w}_{c}",
                                 tag="ot")
                    cw = 512 * nblk_w
                    nc.vector.tensor_scalar_add(ot[:, 0:cw // 2],
                                                ps4[:, 0:cw // 2],
                                                bcol_sb[:])
                    nc.scalar.activation(
                        ot[:, cw // 2:cw], ps4[:, cw // 2:cw],
                        mybir.ActivationFunctionType.Identity,
                        bias=bcol_sb[:])
                    # Stores: one 3-level-AP DMA per col-group quarter g
                    # (partition 32g+m, col 512b+n -> y[26(4w+b)+m,
                    # 2048c+512g+n]), alternating ACT/SP rings.
                    for g in range(4):
                        src_ap = ot[32 * g:32 * g + BK, :]
                        src3 = AP(src_ap.tensor, src_ap.offset,
                                  [[4 * 512, BK], [512, nblk_w], [1, 512]])
                        dst3 = AP(y[:, :].tensor,
                                  WSTEP * w * W + CHW * c + 512 * g,
                                  [[W, BK], [BK * W, nblk_w], [1, 512]])
                        eng = nc.scalar if g % 2 == 0 else nc.sync
                        eng.dma_start(dst3, src3)
    nc.compile()
    return nc


def _get_nc(dtype_key: str):
    if dtype_key not in _NC_CACHE:
        _NC_CACHE[dtype_key] = _build_nc_t32()
    return _NC_CACHE[dtype_key]


def _build_bands_t32(weight: np.ndarray) -> np.ndarray:
    """bands[32b + k, 32j + m] = weight[k-m, j], 0 <= k-m < KH, m < 26."""
    bands = np.zeros((128, KW * 32), dtype=np.float32)
    m = np.arange(BK)
    for j in range(KW):
        for d in range(KH):
            bands[m + d, 32 * j + m] = np.float32(weight[d, j])
    for b in range(1, 4):
        bands[32 * b:32 * b + 32, :] = bands[0:32, :]
    return bands


def kernel(x: np.ndarray, weight: np.ndarray, bias: np.ndarray) -> np.ndarray:
    global LAST_RESULTS
    trace = os.environ.get("CONV_TRACE", "") == "1"

    xs = np.asarray(x, dtype=np.float32)
    assert xs.shape == (H, W), xs.shape
    wf = np.asarray(weight, dtype=np.float32)
    bands = _build_bands_t32(wf).astype(np.float16)
    bcol = np.full((128, 1), np.float32(np.asarray(bias).reshape(-1)[0]),
                   dtype=np.float32)

    xpad = np.zeros((NCORES * RPC + KH - 1, W), dtype=np.float16)
    xpad[:H, :] = xs.astype(np.float16)
    in_maps = []
    for c in range(NCORES):
        xc = np.ascontiguousarray(xpad[c * RPC:c * RPC + IRPC, :])
        in_maps.append({"x": xc, "bands": bands, "bcol": bcol})

    nc = _get_nc("t32")
    kwargs = {}
    if trace:
        kwargs = dict(trace=True, trace_cores=[0])
    res = run_bass_kernel_spmd(nc, in_maps, core_ids=list(range(NCORES)),
                               **kwargs)
    LAST_RESULTS = res
    out = np.concatenate([r["y"][:RPC] for r in res.results], axis=0)[:OH, :OW]
    return np.ascontiguousarray(out.astype(np.float32))
